# revision 6
# baseline (speedup 1.0000x reference)
"""Attention2d Trainium2 kernel.

Sharding: 16 heads / 8 cores = 2 heads per core, data-parallel over all 4
batches on every core (head sharding minimizes rel_pos traffic: each core
reads only its 2 heads' [N, N] slices). The output projection contracts over
all heads' channels, so each core emits a partial projection output over its
64 channels; the host sums the 8 partials and adds b_proj.

Device pipeline per (batch, head) pair:
  qkv     = wqkv^T @ x_b           (PE, one [96, N] psum group; q pre-scaled;
                                    DVE bias eviction)
  v^T     via PE transpose in-place at partitions 64-95 (shifted identity)
  k, q    replicated across 32-partition row groups by SBUF->SBUF DMA so the
          K=32 score matmuls can be packed 4-wide with tile_position row
          tiling (32x128 mode: 4 concurrent matmuls, one PSUM bank each)
  scores  4 rounds; round r computes S^T for chunks jc=r and jc=r+4, both
          512-wide i halves, as 4 concurrent row-tiled matmuls into one
          [128, 2, 1024] psum tile (4 banks)
  p       one ACT exp over the whole 2048-wide round (amortizes the 352cy
          ACT instruction overhead), then *exp(R^T) on DVE (2x fp16 mode);
          one jc chunk per pair multiplied on GpSimd to shorten the DVE queue
  att     [vt|1]^T @ p accumulated over rounds (PE 128x128 mode, F=512,
          i halves in PE output quadrants 0 and 64 of one psum bank; the
          ones column makes rows 32/96 the softmax denominators)
  rcp     = 1 / bcast(colsum)      (ACT row copy -> PE ones-broadcast ->
                                    DVE reciprocal)
  att_sb  = att * rcp              (DVE multiply doubles as psum eviction;
                                    head 0 writes straight into the proj rhs
                                    tile, head 1 is DMA-shifted so rows hold
                                    [h0|h1] x [i-half0|i-half1] stacked)
  out_b  += wp64^T @ att           (PE 64x128 mode: K=64 contracts both
                                    heads at once, 2 concurrent row tiles
                                    handle the two i halves)
"""

import sys

sys.path.insert(0, "/opt/trn_rl_repo")

import numpy as np
import ml_dtypes

import concourse.bass as bass
import concourse.tile as tile
from concourse import mybir, bacc
from concourse.bass_utils import run_bass_kernel_spmd
from concourse.masks import make_identity

B, C = 4, 512
N = 1024  # 32*32 pixels
HEADS, DH = 16, 32
NCORES = 8
HPC = HEADS // NCORES  # heads per core
F16 = mybir.dt.float16
F32 = mybir.dt.float32
AF = mybir.ActivationFunctionType
OP = mybir.AluOpType

_BUILT = None


def build_nc():
    nc = bacc.Bacc("TRN2", target_bir_lowering=False, debug=False, num_devices=NCORES)
    x16 = nc.declare_dram_parameter("x16", [B, C, N], F16, isOutput=False)
    wqkvT = nc.declare_dram_parameter("wqkvT", [C, HPC, 96], F16, isOutput=False)
    bqkv = nc.declare_dram_parameter("bqkv", [96, HPC], F32, isOutput=False)
    wp64T = nc.declare_dram_parameter("wp64T", [128, 4, 128], F16, isOutput=False)
    rt = nc.declare_dram_parameter("rt", [HPC, N, N], F16, isOutput=False)
    outp = nc.declare_dram_parameter("outp", [B, C, N], F16, isOutput=True)

    with tile.TileContext(nc) as tc:
        with (
            tc.tile_pool(name="singles", bufs=1) as singles,
            tc.tile_pool(name="work", bufs=2) as work,
            tc.tile_pool(name="ps", bufs=1, space="PSUM") as pspool,
        ):
            # ---- preamble: constants + resident tensors ----
            idv = singles.tile([96, 32], F16)
            make_identity(nc, idv[64:96, :])
            ones_bc = singles.tile([128, 32], F16)
            nc.vector.memset(ones_bc[:], 1.0)

            wq_sb = singles.tile([128, 4, HPC, 96], F16)
            nc.sync.dma_start(
                wq_sb[:], wqkvT.rearrange("(cc p) h m -> p cc h m", p=128)
            )
            bq_sb = singles.tile([96, HPC], F32)
            nc.sync.dma_start(bq_sb[:], bqkv[:])
            # wp64: rows 0-63 = [h0 d | h1 d] per-core proj weight columns,
            # rows 64-127 a replica for the second 64x128 row tile
            wp_sb = singles.tile([128, 4, 128], F16)
            nc.sync.dma_start(wp_sb[:], wp64T[:])

            # input DMAs ordered/split so the first pair can start early
            xb_sb = singles.tile([128, B, 4, N], F16)
            expRT = [
                singles.tile([128, 2, 4, N], F16, tag=f"expRT{h}", name=f"expRT{h}")
                for h in range(HPC)
            ]
            x16r = x16.rearrange("b (cc p) n -> b p cc n", p=128)
            for cc in range(4):
                nc.sync.dma_start(xb_sb[:, 0, cc], x16r[0, :, cc])
            rtr = rt.rearrange("h (jc p) i -> h p jc i", p=128)
            for h in range(HPC):
                for c in range(2):
                    for r in range(4):
                        nc.sync.dma_start(expRT[h][:, c, r], rtr[h, :, 4 * c + r])
            for b in range(1, B):
                nc.sync.dma_start(xb_sb[:, b], x16r[b])

            # ---- per-pair stage emitters ----
            def emit_qkv_half(b, h, qkv_hold, nn):
                ps_qkv = pspool.tile([96, 512], F32, tag="qps", bufs=2)
                for cc in range(4):
                    nc.tensor.matmul(
                        ps_qkv[:],
                        lhsT=wq_sb[:, cc, h, :],
                        rhs=xb_sb[:, b, cc, 512 * nn : 512 * nn + 512],
                        start=(cc == 0),
                        stop=(cc == 3),
                    )
                nc.vector.tensor_scalar(
                    out=qkv_hold[:, 4 * nn : 4 * nn + 4, :],
                    in0=ps_qkv[:],
                    scalar1=bq_sb[:, h : h + 1],
                    scalar2=None,
                    op0=OP.add,
                )

            def emit_repl(qkv_hold):
                # replicate q to row groups 1-3 and k chunk-halves to row
                # groups 0-1 (chunks 0-3) / 2-3 (chunks 4-7) for row tiling
                q4 = work.tile([128, 8, 128], F16, tag="q4")
                kk = work.tile([128, 4, 128], F16, tag="kk")
                for g in range(1, 4):
                    nc.sync.dma_start(q4[32 * g : 32 * g + 32], qkv_hold[0:32])
                for g in range(4):
                    nc.gpsimd.dma_start(
                        kk[32 * g : 32 * g + 32],
                        qkv_hold[32:64, 4 * (g // 2) : 4 * (g // 2) + 4, :],
                    )
                return q4, kk

            def emit_vt(qkv_hold):
                vt_ps = pspool.tile([128, 8, 32], F16, tag="vb")
                for jc in range(8):
                    nc.tensor.transpose(
                        vt_ps[:, jc, :],
                        qkv_hold[64:96, jc, :],
                        idv[64:96, :],
                    )
                vt1 = work.tile([128, 8, 34], F16, tag="vt1")
                nc.vector.tensor_copy(vt1[:, :, 0:32], vt_ps[:])
                nc.vector.memset(vt1[:, :, 32:33], 1.0)
                return vt1

            def emit_mid(b, h, qkv_hold, q4, kk, vt1, deferred, fin_late, inject):
                # 4 score rounds; round r covers chunks (r, r+4) x both
                # i-halves as 4 concurrent row-tiled matmuls -> 1 ACT exp ->
                # DVE/GpSimd rel-pos multiply. attv lags one round so the PE
                # never waits on the exp chain; deferred proj chunks and the
                # next pair's qkv fill remaining PE gaps.
                p2 = work.tile([128, 2, 4, N], F16, tag="p2")
                att_ps = pspool.tile([128, 512], F32, tag="att", bufs=1)

                def attv(r):
                    for cg in range(4):
                        c, nn = cg // 2, cg % 2
                        jc = 4 * c + r
                        nc.tensor.matmul(
                            att_ps[64 * nn : 64 * nn + 33, :],
                            lhsT=vt1[:, jc, 0:33],
                            rhs=p2[:, c, r, 512 * nn : 512 * nn + 512],
                            start=(jc == 0),
                            stop=(jc == 7),
                        )

                for r in range(4):
                    sc_ps = pspool.tile([128, 2, N], F32, tag="sc", bufs=1)
                    for g in range(4):
                        c, nn = g // 2, g % 2
                        nc.tensor.matmul(
                            sc_ps[:, c, 512 * nn : 512 * nn + 512],
                            lhsT=kk[32 * g : 32 * g + 32, r, :],
                            rhs=(qkv_hold if g == 0 else q4)[
                                32 * g : 32 * g + 32, 4 * nn : 4 * nn + 4, :
                            ],
                            start=True,
                            stop=True,
                            tile_position=(32 * g, 0),
                        )
                    nc.scalar.activation(p2[:, :, r, :], sc_ps[:], AF.Exp)
                    if r >= 1:
                        attv(r - 1)
                    # GpSimd takes one chunk of round 2 (lag gives it slack)
                    if r == 2:
                        nc.vector.tensor_tensor(
                            p2[:, 0, r, :],
                            p2[:, 0, r, :],
                            expRT[h][:, 0, r, :],
                            OP.mult,
                        )
                        nc.gpsimd.tensor_tensor(
                            p2[:, 1, r, :],
                            p2[:, 1, r, :],
                            expRT[h][:, 1, r, :],
                            OP.mult,
                        )
                    else:
                        nc.vector.tensor_tensor(
                            p2[:, :, r, :],
                            p2[:, :, r, :],
                            expRT[h][:, :, r, :],
                            OP.mult,
                        )
                    if r == 0 and fin_late is not None:
                        fin_late()
                    if r == 1 and inject:
                        inject.pop(0)()
                    if r == 2 and inject:
                        inject.pop(0)()
                    if r >= 2 and deferred:
                        deferred.pop(0)()
                attv(3)
                if deferred:
                    deferred.pop(0)()
                return att_ps

            def emit_fin_early(att_ps):
                # denominators: rows 32/96 of att_ps -> sbuf
                cs = work.tile([128, 512], F16, tag="cs")
                for nn in range(2):
                    rr = 64 * nn + 32
                    nc.scalar.activation(
                        cs[rr : rr + 1, :], att_ps[rr : rr + 1, :], AF.Identity
                    )
                return cs

            def emit_fin_late(att_ps, cs, dst, dst_half):
                # ones-broadcast across quadrant partitions -> reciprocal ->
                # fused evict-mult into dst (proj rhs for h0, staging for h1)
                bc_ps = pspool.tile([128, 512], F32, tag="qps", bufs=2)
                for nn in range(2):
                    rr = 64 * nn + 32
                    nc.tensor.matmul(
                        bc_ps[64 * nn : 64 * nn + 32, :],
                        lhsT=ones_bc[rr : rr + 1, 0:32],
                        rhs=cs[rr : rr + 1, :],
                        start=True,
                        stop=True,
                        tile_position=(rr, 64 * nn),
                    )
                rcp = work.tile([128, 512], F32, tag="rcp")
                nc.vector.reciprocal_approx_fast(rcp[:], bc_ps[:])
                nc.vector.tensor_tensor(dst[:], att_ps[:], rcp[:], OP.mult)
                if dst_half is not None:
                    # h1: shift quadrants down 32 partitions into the proj rhs
                    nc.sync.dma_start(dst_half[32:64], dst[0:32])
                    nc.sync.dma_start(dst_half[96:128], dst[64:96])

            def make_proj(b, proj_rhs):
                # 4 deferred chunks; each: two concurrent 64x128 row-tiled
                # matmuls (K=64 contracts both heads), eviction, output DMA
                out_sb = work.tile([128, 4, N], F16, tag="out_sb")
                outr = outp[b].rearrange("(oc p) n -> p oc n", p=128)

                def chunk(oc):
                    def run():
                        ps = [
                            pspool.tile(
                                [128, 512], F32, tag="qps", bufs=2, name=f"ps_o{i}"
                            )
                            for i in range(2)
                        ]
                        for nn in range(2):
                            nc.tensor.matmul(
                                ps[nn][:],
                                lhsT=wp_sb[64 * nn : 64 * nn + 64, oc, :],
                                rhs=proj_rhs[64 * nn : 64 * nn + 64, :],
                                start=True,
                                stop=True,
                                tile_position=(64 * nn, 0),
                            )
                        for nn in range(2):
                            dst = out_sb[:, oc, 512 * nn : 512 * nn + 512]
                            if oc % 2 == 0:
                                nc.scalar.activation(dst, ps[nn][:], AF.Identity)
                            else:
                                nc.vector.tensor_copy(dst, ps[nn][:])
                        nc.gpsimd.dma_start(outr[:, oc], out_sb[:, oc, :])

                    return run

                return [chunk(oc) for oc in range(4)]

            # ---- main loop, software-pipelined across pairs ----
            pairs = [(b, h) for b in range(B) for h in range(HPC)]
            qkv_tiles = {}
            proj_tiles = {}
            deferred = []

            def make_qkv_inject(idx):
                b, h = pairs[idx]

                def half0():
                    qkv_hold = work.tile([96, 8, 128], F16, tag="qkv_hold")
                    qkv_tiles[idx] = [qkv_hold, None, None]
                    emit_qkv_half(b, h, qkv_hold, 0)

                def half1():
                    qkv_hold = qkv_tiles[idx][0]
                    emit_qkv_half(b, h, qkv_hold, 1)
                    q4, kk = emit_repl(qkv_hold)
                    qkv_tiles[idx][1] = q4
                    qkv_tiles[idx][2] = kk

                return [half0, half1]

            def make_fin_late(pidx, pb, ph, patt, cs):
                def run():
                    if ph == 0:
                        proj_rhs = work.tile([128, 512], F16, tag="proj_rhs")
                        proj_tiles[pb] = proj_rhs
                        emit_fin_late(patt, cs, proj_rhs, None)
                    else:
                        proj_rhs = proj_tiles.pop(pb)
                        att_h1 = work.tile([128, 512], F16, tag="att_h1")
                        emit_fin_late(patt, cs, att_h1, proj_rhs)
                        deferred.extend(make_proj(pb, proj_rhs))

                return run

            for fn in make_qkv_inject(0):
                fn()
            pending = None  # (idx, b, h, att_ps)
            fin_late = None
            for idx, (b, h) in enumerate(pairs):
                qkv_hold, q4, kk = qkv_tiles.pop(idx)
                vt1 = emit_vt(qkv_hold)
                if pending is not None:
                    pidx, pb, ph, patt = pending
                    cs = emit_fin_early(patt)
                    fin_late = make_fin_late(pidx, pb, ph, patt, cs)
                inject = make_qkv_inject(idx + 1) if idx + 1 < len(pairs) else []
                pending = (
                    idx,
                    b,
                    h,
                    emit_mid(b, h, qkv_hold, q4, kk, vt1, deferred, fin_late, inject),
                )
                fin_late = None
            pidx, pb, ph, patt = pending
            cs = emit_fin_early(patt)
            make_fin_late(pidx, pb, ph, patt, cs)()
            for fn in deferred:
                fn()

    nc.compile()
    return nc


def _get_nc():
    global _BUILT
    if _BUILT is None:
        _BUILT = build_nc()
    return _BUILT


def _prep_inputs(x, w_qkv, b_qkv, w_proj, b_proj, shared_rel_pos):
    """Host-side sharding/layout prep. Returns per-core input maps."""
    scale = np.float32(DH**-0.5)
    x16 = np.ascontiguousarray(x.reshape(B, C, N)).astype(np.float16)

    wq = w_qkv.reshape(HEADS, 96, C).astype(np.float32).copy()
    wq[:, 0:32, :] *= scale  # fold attention scale into q
    bq = b_qkv.reshape(HEADS, 96).astype(np.float32).copy()
    bq[:, 0:32] *= scale

    in_maps = []
    for g in range(NCORES):
        hh = [HPC * g + h for h in range(HPC)]
        wqkvT = np.ascontiguousarray(
            wq[hh].transpose(2, 0, 1).astype(np.float16)
        )  # [C, HPC, 96]
        bqkv = np.ascontiguousarray(bq[hh].T)  # [96, HPC]
        # proj weight columns for this core's heads: [64 (h d), 4 oc, 128 c']
        # stacked twice along partitions for the two 64x128 row tiles
        wp = w_proj[:, 64 * g : 64 * (g + 1)].astype(np.float32)  # [C, 64]
        wp64 = wp.T.reshape(64, 4, 128)
        wp64T = np.ascontiguousarray(
            np.concatenate([wp64, wp64], axis=0).astype(np.float16)
        )
        rt = np.ascontiguousarray(
            np.exp(shared_rel_pos[0, hh].transpose(0, 2, 1).astype(np.float32))
        ).astype(np.float16)  # [HPC, N, N] = exp(R^T) per head
        in_maps.append(
            {"x16": x16, "wqkvT": wqkvT, "bqkv": bqkv, "wp64T": wp64T, "rt": rt}
        )
    return in_maps


def kernel(x, w_qkv, b_qkv, w_proj, b_proj, shared_rel_pos, _trace=False):
    nc = _get_nc()
    in_maps = _prep_inputs(x, w_qkv, b_qkv, w_proj, b_proj, shared_rel_pos)
    res = run_bass_kernel_spmd(nc, in_maps, list(range(NCORES)), trace=_trace)
    kernel.last_result = res
    out = np.zeros((B, C, N), np.float32)
    for g in range(NCORES):
        out += res.results[g]["outp"].astype(np.float32)
    out += b_proj.astype(np.float32)[None, :, None]
    return out.reshape(B, C, 32, 32).astype(np.float32)
